# revision 1
# baseline (speedup 1.0000x reference)
"""Damped electrostatics (shifted force) TRN2 kernel.

Strategy:
  - Shard the edge dimension E=3.2M across 8 NeuronCores (400K edges each).
  - Host marshals inputs: gathers per-atom records (charges/dipoles/quadrupoles)
    to per-edge streams with np.take (pure data movement), reshapes each core's
    edges to a [128, 3200] partition-major layout (3125 real cols + padding),
    interleaved as a hot stream [d, v, qu, qv] and a cold stream [du, dv, Q9]
    (Q9 column-permuted to [diag, upper, lower]).
  - Device computes the full physics pipeline: the GPSIMD (Pool) engine forms
    all raw products (v*dip, du*dv, outer(v), vv*Q) from DMA'd tiles only; the
    DVE does the chi/switch chain, slice-add reductions and assembly with
    1/d, 1/d^2 folded into the coefficients; ACT does squares/sqrt.

Self-contained: hardcodes all shapes; no file reads.
"""
import numpy as np

import concourse.bass as bass
import concourse.bacc as bacc
import concourse.tile as tile
from concourse import mybir
from concourse.bass_utils import run_bass_kernel_spmd

F32 = mybir.dt.float32

N_CORES = 8
E_TOTAL = 3_200_000
E_CORE = E_TOTAL // N_CORES      # 400_000
P = 128
COLS_REAL = E_CORE // P          # 3125
COLS = 3200                      # padded
K = 320                          # tile columns
NT = COLS // K                   # 10 tiles

CUTOFF = 10.0
CUTOFF_SR = 4.0
KEHALF = 7.199822675975274

_CACHE = {}


def _ap(t, ap_dims):
    return bass.AP(tensor=t.tensor, offset=t.offset, ap=ap_dims)


def _bcast_inner(t_ap, n):
    """Append a broadcast (step 0) innermost dim of size n."""
    return bass.AP(tensor=t_ap.tensor, offset=t_ap.offset, ap=[*t_ap.ap, [0, n]])


def _build(cols=COLS, passes=1, ablate=(), loop_n=0):
    ablate = frozenset(ablate)
    nc = bacc.Bacc("TRN2", target_bir_lowering=False, debug=False,
                   num_devices=N_CORES)
    A = mybir.AluOpType
    AF = mybir.ActivationFunctionType

    # tile-blocked planar streams: [P, n_tiles, w, K]
    # hot [d, vx, vy, vz, uq, vq]; cold [ud3, vd3, Qdiag, Qup, Qlo]
    nt = cols // K
    s6 = nc.dram_tensor("s6_in", [P, nt, 6, K], F32, kind="ExternalInput")
    s15 = nc.dram_tensor("s15_in", [P, nt, 15, K], F32, kind="ExternalInput")
    eout = nc.dram_tensor("eout", [P, cols], F32, kind="ExternalOutput")

    with tile.TileContext(nc) as tc:
        with tc.tile_pool(name="io", bufs=2) as io, \
             tc.tile_pool(name="tp", bufs=2) as tp, \
             tc.tile_pool(name="cst", bufs=1) as cst:
            bias_t = cst.tile([P, 4], F32)
            for i, bv in enumerate([-1.0, -0.2, -0.03, -0.004]):
                nc.vector.memset(bias_t[:, i:i + 1], bv)
            dummy6 = None
            if "pool" in ablate:
                dummy6 = cst.tile([P, 6, K], F32)
                nc.vector.memset(dummy6[:], 0.5)

            def load(it):
                S = {}
                st = io.tile([P, 6, K], F32, name="st")
                nc.sync.dma_start(out=st[:], in_=s6[:, it, :, :])
                sc = io.tile([P, 15, K], F32, name="sc")
                nc.sync.dma_start(out=sc[:], in_=s15[:, it, :, :])
                S["it"] = it
                S["st"], S["sc"] = st, sc
                S["d"] = st[:, 0, :]
                S["v"] = st[:, 1:4, :]
                S["uq"] = st[:, 4, :]
                S["vq"] = st[:, 5, :]
                S["ud"] = sc[:, 0:3, :]
                S["vd"] = sc[:, 3:6, :]
                S["qdiag"] = sc[:, 6:9, :]
                S["qup"] = sc[:, 9:12, :]
                S["qlo"] = sc[:, 12:15, :]
                return S

            def stage_pool(S):
                st, v_t, ud_t, vd_t = S["st"], S["v"], S["ud"], S["vd"]
                p6 = tp.tile([P, 6, K], F32, name="p6")
                nc.gpsimd.tensor_mul(out=p6[:, 0:3, :], in0=ud_t, in1=v_t)
                nc.gpsimd.tensor_mul(out=p6[:, 3:6, :], in0=vd_t, in1=v_t)
                p3c = tp.tile([P, 3, K], F32, name="p3c")
                nc.gpsimd.tensor_mul(out=p3c[:], in0=ud_t, in1=vd_t)
                vv6 = tp.tile([P, 6, K], F32, name="vv6")
                nc.gpsimd.tensor_mul(out=vv6[:, 0:3, :], in0=v_t, in1=v_t)
                vx = st[:, 1, :]
                vxb = _ap(vx, [vx.ap[0], [0, 2], [1, K]])
                nc.gpsimd.tensor_tensor(out=vv6[:, 3:5, :], in0=vxb,
                                        in1=st[:, 2:4, :], op=A.mult)
                nc.gpsimd.tensor_mul(out=vv6[:, 5, :], in0=st[:, 2, :],
                                     in1=st[:, 3, :])
                qoff = tp.tile([P, 3, K], F32, name="qoff")
                nc.gpsimd.tensor_tensor(out=qoff[:], in0=S["qup"], in1=S["qlo"],
                                        op=A.add)
                pq6 = tp.tile([P, 6, K], F32, name="pq6")
                nc.gpsimd.tensor_mul(out=pq6[:, 0:3, :], in0=vv6[:, 0:3, :],
                                     in1=S["qdiag"])
                nc.gpsimd.tensor_mul(out=pq6[:, 3:6, :], in0=vv6[:, 3:6, :],
                                     in1=qoff[:])
                S["p6"], S["p3c"], S["pq6"] = p6, p3c, pq6

            def stage_chain(S):
                d_t = S["d"]
                T_invd = tp.tile([P, K], F32, name="T_invd")
                T_a = tp.tile([P, K], F32, name="T_a")
                T_x = tp.tile([P, K], F32, name="T_x")
                T_b = tp.tile([P, K], F32, name="T_b")
                T_c = tp.tile([P, K], F32, name="T_c")
                T_d = tp.tile([P, K], F32, name="T_d")
                T_e = tp.tile([P, K], F32, name="T_e")
                T_f = tp.tile([P, K], F32, name="T_f")
                T_g = tp.tile([P, K], F32, name="T_g")
                nc.vector.reciprocal(out=T_invd[:], in_=d_t)            # 1/d
                nc.vector.tensor_mul(out=T_a[:], in0=d_t, in1=d_t)
                nc.scalar.activation(out=T_a[:], in_=T_a[:], func=AF.Sqrt,
                                     bias=1.0, scale=1.0)
                nc.vector.reciprocal(out=T_a[:], in_=T_a[:])            # ddinv
                nc.vector.tensor_scalar(out=T_x[:], in0=d_t, scalar1=CUTOFF_SR,
                                        scalar2=1.0 / CUTOFF_SR, op0=A.min,
                                        op1=A.mult)                     # x
                nc.vector.tensor_mul(out=T_b[:], in0=T_x[:], in1=T_x[:])
                nc.vector.tensor_mul(out=T_b[:], in0=T_x[:], in1=T_b[:])  # x3
                nc.vector.tensor_scalar(out=T_c[:], in0=T_x[:], scalar1=6.0,
                                        scalar2=15.0, op0=A.mult,
                                        op1=A.subtract)
                nc.vector.tensor_mul(out=T_c[:], in0=T_c[:], in1=T_x[:])
                nc.vector.scalar_tensor_tensor(out=T_c[:], in0=T_c[:],
                                               scalar=10.0, in1=T_b[:],
                                               op0=A.add, op1=A.mult)   # t3
                nc.vector.tensor_scalar(out=T_c[:], in0=T_c[:], scalar1=1.0,
                                        scalar2=None, op0=A.subtract)   # nsw
                nc.vector.tensor_sub(out=T_a[:], in0=T_invd[:], in1=T_a[:])
                nc.vector.tensor_mul(out=T_a[:], in0=T_c[:], in1=T_a[:])
                nc.vector.tensor_add(out=T_a[:], in0=T_a[:], in1=T_invd[:])  # chi
                nc.vector.tensor_mul(out=T_d[:], in0=T_a[:], in1=T_a[:])
                nc.vector.tensor_mul(out=T_c[:], in0=T_d[:], in1=T_a[:])  # chi3
                nc.vector.scalar_tensor_tensor(out=T_e[:], in0=d_t, scalar=0.01,
                                               in1=T_a[:], op0=A.mult,
                                               op1=A.add)
                nc.vector.tensor_scalar(out=T_e[:], in0=T_e[:], scalar1=0.2,
                                        scalar2=None, op0=A.subtract)   # Ac
                nc.vector.scalar_tensor_tensor(out=T_f[:], in0=d_t, scalar=0.002,
                                               in1=T_d[:], op0=A.mult,
                                               op1=A.add)
                nc.vector.tensor_scalar(out=T_f[:], in0=T_f[:], scalar1=0.03,
                                        scalar2=None, op0=A.subtract)   # Bc
                nc.vector.scalar_tensor_tensor(out=T_c[:], in0=d_t, scalar=0.0003,
                                               in1=T_c[:], op0=A.mult,
                                               op1=A.add)
                nc.vector.tensor_scalar(out=T_c[:], in0=T_c[:], scalar1=0.004,
                                        scalar2=None, op0=A.subtract)   # Cc
                nc.vector.tensor_mul(out=T_d[:], in0=T_invd[:], in1=T_invd[:])
                nc.vector.tensor_mul(out=T_f[:], in0=T_f[:], in1=T_invd[:])  # Bd
                nc.vector.tensor_mul(out=T_g[:], in0=T_c[:], in1=T_d[:])     # Cd2
                S["Ac"], S["Bd"], S["Cc"], S["Cd2"] = T_e, T_f, T_c, T_g

            def stage_out(S):
                d_t, uq_t, vq_t = S["d"], S["uq"], S["vq"]
                T_e, T_f, T_c, T_g = S["Ac"], S["Bd"], S["Cc"], S["Cd2"]
                T_m = tp.tile([P, K], F32, name="T_m")
                T_i = tp.tile([P, K], F32, name="T_i")
                T_k = tp.tile([P, K], F32, name="T_k")
                T_l = tp.tile([P, K], F32, name="T_l")
                if "pool" in ablate:
                    p6 = p3c = pq6 = None
                    dot2 = tp.tile([P, 2, K], F32, name="dot2")
                    nc.vector.tensor_add(out=dot2[:], in0=dummy6[:, 0:2, :],
                                         in1=dummy6[:, 2:4, :])
                    nc.vector.tensor_add(out=dot2[:], in0=dot2[:],
                                         in1=dummy6[:, 4:6, :])
                    nc.vector.tensor_add(out=T_i[:], in0=dummy6[:, 0, :],
                                         in1=dummy6[:, 1, :])
                    nc.vector.tensor_add(out=T_i[:], in0=T_i[:],
                                         in1=dummy6[:, 2, :])
                    h3 = tp.tile([P, 3, K], F32, name="h3")
                    nc.vector.tensor_add(out=h3[:], in0=dummy6[:, 0:3, :],
                                         in1=dummy6[:, 3:6, :])
                else:
                    p6, p3c, pq6 = S["p6"], S["p3c"], S["pq6"]
                    dot2 = tp.tile([P, 2, K], F32, name="dot2")
                    b = p6[:]
                    a0 = _ap(b, [b.ap[0], [3 * K, 2], [1, K]])
                    a1 = bass.AP(tensor=b.tensor, offset=b.offset + K,
                                 ap=[b.ap[0], [3 * K, 2], [1, K]])
                    a2 = bass.AP(tensor=b.tensor, offset=b.offset + 2 * K,
                                 ap=[b.ap[0], [3 * K, 2], [1, K]])
                    nc.vector.tensor_add(out=dot2[:], in0=a0, in1=a1)
                    nc.vector.tensor_add(out=dot2[:], in0=dot2[:], in1=a2)
                    nc.vector.tensor_add(out=T_i[:], in0=p3c[:, 0, :],
                                         in1=p3c[:, 1, :])
                    nc.vector.tensor_add(out=T_i[:], in0=T_i[:],
                                         in1=p3c[:, 2, :])                # dipdot
                    h3 = tp.tile([P, 3, K], F32, name="h3")
                    nc.vector.tensor_add(out=h3[:], in0=pq6[:, 0:3, :],
                                         in1=pq6[:, 3:6, :])
                dvur = dot2[:, 0, :]
                duvr = dot2[:, 1, :]
                nc.vector.tensor_add(out=T_k[:], in0=h3[:, 0, :], in1=h3[:, 1, :])
                nc.vector.tensor_add(out=T_k[:], in0=T_k[:], in1=h3[:, 2, :])
                nc.vector.tensor_add(out=T_l[:], in0=S["qdiag"][:, 0, :],
                                     in1=S["qdiag"][:, 1, :])
                nc.vector.tensor_add(out=T_l[:], in0=T_l[:],
                                     in1=S["qdiag"][:, 2, :])             # trq
                nc.vector.tensor_mul(out=T_e[:], in0=vq_t, in1=T_e[:])    # e1
                nc.vector.tensor_mul(out=T_m[:], in0=duvr, in1=T_f[:])    # e2
                nc.vector.scalar_tensor_tensor(out=T_e[:], in0=T_m[:],
                                               scalar=2.0, in1=T_e[:],
                                               op0=A.mult, op1=A.add)     # e12
                nc.vector.tensor_mul(out=T_k[:], in0=T_k[:], in1=T_g[:])  # z
                nc.vector.scalar_tensor_tensor(out=T_l[:], in0=T_l[:],
                                               scalar=1.0 / 3.0, in1=T_c[:],
                                               op0=A.mult, op1=A.mult)    # y
                nc.vector.tensor_sub(out=T_k[:], in0=T_k[:], in1=T_l[:])  # zy
                nc.vector.tensor_add(out=T_e[:], in0=T_e[:], in1=T_k[:])  # e123
                nc.vector.tensor_mul(out=T_e[:], in0=T_e[:], in1=uq_t)    # eu
                nc.vector.tensor_mul(out=T_m[:], in0=duvr, in1=dvur)      # tt
                nc.vector.tensor_mul(out=T_m[:], in0=T_m[:], in1=T_g[:])  # m2
                nc.vector.tensor_mul(out=T_i[:], in0=T_i[:], in1=T_c[:])  # m1
                nc.vector.scalar_tensor_tensor(out=T_i[:], in0=T_m[:],
                                               scalar=3.0, in1=T_i[:],
                                               op0=A.mult, op1=A.subtract)
                nc.vector.tensor_sub(out=T_e[:], in0=T_e[:], in1=T_i[:])  # Ee
                nc.vector.tensor_scalar(out=T_l[:], in0=d_t, scalar1=CUTOFF,
                                        scalar2=None, op0=A.is_le)        # mask
                out_t = io.tile([P, K], F32, name="out_t")
                nc.vector.scalar_tensor_tensor(out=out_t[:], in0=T_e[:],
                                               scalar=KEHALF, in1=T_l[:],
                                               op0=A.mult, op1=A.mult)
                nc.scalar.dma_start(out=eout[:, S["it"] * K:(S["it"] + 1) * K],
                                  in_=out_t[:])

            import contextlib
            loop_cm = tc.For_i(0, loop_n, 1) if loop_n else contextlib.nullcontext()
            with loop_cm:
                tiles = [t % nt for t in range(passes * nt)]
                prev = None
                for idx in range(len(tiles) + 1):
                    S = None
                    if idx < len(tiles):
                        it = tiles[idx]
                        s = slice(it * K, (it + 1) * K)
                        S = load(it)
                        if "math" in ablate:
                            out_t = io.tile([P, K], F32, name="out_t")
                            nc.vector.tensor_add(out=out_t[:], in0=S["d"],
                                                 in1=S["uq"])
                            nc.vector.tensor_add(out=out_t[:], in0=out_t[:],
                                                 in1=S["sc"][:, 0, :])
                            nc.scalar.dma_start(out=eout[:, s], in_=out_t[:])
                            S = None
                        elif "dve" in ablate:
                            stage_pool(S)
                            out_t = io.tile([P, K], F32, name="out_t")
                            nc.gpsimd.tensor_copy(out=out_t[:],
                                                  in_=S["pq6"][:, 0, :])
                            nc.scalar.dma_start(out=eout[:, s], in_=out_t[:])
                            S = None
                        else:
                            if "pool" not in ablate:
                                stage_pool(S)
                            stage_chain(S)
                    if prev is not None:
                        stage_out(prev)
                    prev = S
    nc.compile()
    return nc


def _pack(parts, w):
    """parts: list of [E_CORE(,k)] arrays -> planar [P, w, COLS] f32."""
    out = np.zeros((P, w, COLS), np.float32)
    pl = 0
    for a in parts:
        if a.ndim == 1:
            out[:, pl, :COLS_REAL] = a.reshape(P, COLS_REAL)
            pl += 1
        else:
            k = a.shape[1]
            out[:, pl:pl + k, :COLS_REAL] = np.moveaxis(
                a.reshape(P, COLS_REAL, k), 2, 1)
            pl += k
    assert pl == w
    return out


def kernel(atomic_charges, atomic_dipoles, atomic_quadrupoles,
           vectors_uv, distances_uv, idx_u, idx_v):
    q = np.ascontiguousarray(np.asarray(atomic_charges, np.float32))
    dip = np.ascontiguousarray(np.asarray(atomic_dipoles, np.float32))
    quad = np.ascontiguousarray(
        np.asarray(atomic_quadrupoles, np.float32)).reshape(-1, 9)
    quad = np.ascontiguousarray(quad[:, [0, 4, 8, 1, 2, 5, 3, 6, 7]])
    vec = np.ascontiguousarray(np.asarray(vectors_uv, np.float32))
    dist = np.ascontiguousarray(np.asarray(distances_uv, np.float32))
    iu = np.asarray(idx_u).astype(np.int64)
    iv = np.asarray(idx_v).astype(np.int64)

    if "nc" not in _CACHE:
        _CACHE["nc"] = _build()
    nc = _CACHE["nc"]

    in_maps = []
    for c in range(N_CORES):
        sl = slice(c * E_CORE, (c + 1) * E_CORE)
        iu_c, iv_c = iu[sl], iv[sl]
        d_c = np.where(dist[sl] == 0, 1.0, dist[sl]).astype(np.float32)
        m6 = _pack([d_c, vec[sl], q[iu_c], q[iv_c]], 6)
        m6[:, 0, COLS_REAL:] = 1.0            # pad d -> 1 (avoid 1/0)
        m15 = _pack([dip[iu_c], dip[iv_c], quad[iv_c]], 15)
        # tile-block: [P, w, COLS] -> [P, NT, w, K]
        m6 = np.ascontiguousarray(
            np.moveaxis(m6.reshape(P, 6, NT, K), 2, 1))
        m15 = np.ascontiguousarray(
            np.moveaxis(m15.reshape(P, 15, NT, K), 2, 1))
        in_maps.append({"s6_in": m6, "s15_in": m15})

    res = run_bass_kernel_spmd(nc, in_maps, core_ids=list(range(N_CORES)))
    _CACHE["last_results"] = res

    out = np.empty(E_TOTAL, np.float32)
    for c in range(N_CORES):
        out[c * E_CORE:(c + 1) * E_CORE] = \
            res.results[c]["eout"][:, :COLS_REAL].reshape(-1)
    return out



# revision 12
# speedup vs baseline: 1.4117x; 1.4117x over previous
"""Damped electrostatics (shifted force) TRN2 kernel — v3 (f16, 3-engine).

Strategy:
  - Shard E=3.2M edges across 8 NeuronCores (400K edges each).
  - Host marshals: gathers per-atom records to per-edge streams (np.take,
    pure data movement), casts to f16, packs one planar stream
    s21 = [d, v(3), qu, qv, du(3), dv(3), Qdiag(3), Qup(3), Qlo(3)]
    as [P=128, NT, 21, K] per core (COLS=3200 = 3125 real + pad).
  - Device (per tile): ACT does the unary ops (Square/Sqrt/affine Copy —
    one act-func set); Pool (GPSIMD) does the quadrupole/dipole product
    block via scalar_tensor_tensor (0.6-efficiency path); DVE does the
    rest in f16 fast modes (tensor_tensor 2x, tensor_scalar 4x).
  - Math: chi = ddinv - p*(ddinv - invd), p = x^3(6x^2-15x+10), x=min(d,4)/4;
    E = KEHALF*mask*[qu*(qv*Ac + rawuv*Bd + (vQv - trQ*d2/3)*invd2*Cc)
        + (dipdot - 3*rawuv*rawvu*invd2)*Cc]
    with Ac = chi+0.01d-0.2, Bd = (2chi^2+0.004d-0.06)*invd,
    Cc = chi^3+0.0003d-0.004, rawuv = v.dv, rawvu = v.du.

Self-contained: hardcodes all shapes; no file reads.
"""
import numpy as np

import concourse.bass as bass
import concourse.bacc as bacc
import concourse.tile as tile
from concourse import mybir
from concourse.bass_utils import run_bass_kernel_spmd

F16 = mybir.dt.float16
F32 = mybir.dt.float32

N_CORES = 8
E_TOTAL = 3_200_000
E_CORE = E_TOTAL // N_CORES      # 400_000
P = 128
COLS_REAL = E_CORE // P          # 3125
COLS = 3200                      # padded
NPL = 21                         # input planes per edge

CUTOFF = 10.0
CUTOFF_SR = 4.0
KEHALF = 7.199822675975274
SQRT2 = 1.4142135623730951

_CACHE = {}


def _build(K=640, io_bufs=3, tp_bufs=2):
    NT = COLS // K
    nc = bacc.Bacc("TRN2", target_bir_lowering=False, debug=False,
                   num_devices=N_CORES)
    A = mybir.AluOpType
    AF = mybir.ActivationFunctionType

    s21 = nc.dram_tensor("s21_in", [P, NT, NPL, K], F16, kind="ExternalInput")
    eout = nc.dram_tensor("eout", [P, COLS], F16, kind="ExternalOutput")

    def stt(out, in0, in1, op):
        """Pool binary op (TensorTensor is the only legal Pool ALU opcode)."""
        nc.gpsimd.tensor_tensor(out=out, in0=in0, in1=in1, op=op)

    with tile.TileContext(nc) as tc, \
         nc.allow_low_precision(reason="f16 kernel, rel-err gate 2e-2"):
        with tc.tile_pool(name="io", bufs=io_bufs) as io, \
             tc.tile_pool(name="tp", bufs=tp_bufs) as tp:

            def tile_prog(it):
                st = io.tile([P, NPL, K], F16, name="st")
                nc.sync.dma_start(out=st[:], in_=s21[:, it, :, :])
                d = st[:, 0, :]
                v = st[:, 1:4, :]
                qu = st[:, 4, :]
                qv = st[:, 5, :]
                du = st[:, 6:9, :]
                dv = st[:, 9:12, :]
                Qd = st[:, 12:15, :]
                Qup = st[:, 15:18, :]
                Qlo = st[:, 18:21, :]

                # ---- Pool: product block + trQ (independent of DVE) ----
                dd3 = tp.tile([P, 3, K], F16, name="dd3")
                stt(dd3[:], du, dv, A.mult)
                qoff = tp.tile([P, 3, K], F16, name="qoff")
                stt(qoff[:], Qup, Qlo, A.add)
                vv6 = tp.tile([P, 6, K], F16, name="vv6")
                stt(vv6[:, 0:3, :], v, v, A.mult)
                vx = st[:, 1, :]
                vxb = bass.AP(tensor=vx.tensor, offset=vx.offset,
                              ap=[vx.ap[0], [0, 2], [1, K]])
                nc.vector.tensor_mul(out=vv6[:, 3:5, :], in0=vxb,
                                     in1=st[:, 2:4, :])
                nc.vector.tensor_mul(out=vv6[:, 5, :], in0=st[:, 2, :],
                                     in1=st[:, 3, :])
                pq6 = vv6
                stt(pq6[:, 0:3, :], vv6[:, 0:3, :], Qd, A.mult)
                stt(pq6[:, 3:6, :], vv6[:, 3:6, :], qoff[:], A.mult)
                trQ = tp.tile([P, K], F16, name="trQ")
                nc.vector.tensor_add(out=trQ[:], in0=Qd[:, 0, :],
                                     in1=Qd[:, 1, :])
                nc.vector.tensor_add(out=trQ[:], in0=trQ[:],
                                     in1=Qd[:, 2, :])

                # ---- ACT: unaries from d ----
                d2 = tp.tile([P, K], F16, name="d2")
                nc.scalar.activation(out=d2[:], in_=d, func=AF.Square)
                ddamp = tp.tile([P, K], F16, name="ddamp")
                nc.scalar.activation(out=ddamp[:], in_=d2[:], func=AF.Sqrt,
                                     bias=1.0, scale=1.0)
                s1 = tp.tile([P, K], F16, name="s1")
                nc.scalar.activation(out=s1[:], in_=d, func=AF.Copy,
                                     bias=-0.2, scale=0.01)
                s2 = tp.tile([P, K], F16, name="s2")
                nc.scalar.activation(out=s2[:], in_=d, func=AF.Copy,
                                     bias=-0.06, scale=0.004)
                s3 = tp.tile([P, K], F16, name="s3")
                nc.scalar.activation(out=s3[:], in_=d, func=AF.Copy,
                                     bias=-0.004, scale=0.0003)

                # ---- DVE: st-dependent work first (overlaps ACT/Pool) ----
                p6 = tp.tile([P, 6, K], F16, name="p6")
                nc.vector.tensor_mul(out=p6[:, 0:3, :], in0=v, in1=dv)
                nc.vector.tensor_mul(out=p6[:, 3:6, :], in0=v, in1=du)
                dot2 = tp.tile([P, 2, K], F16, name="dot2")
                b = p6[:]
                a0 = bass.AP(tensor=b.tensor, offset=b.offset,
                             ap=[b.ap[0], [3 * K, 2], [1, K]])
                a1 = bass.AP(tensor=b.tensor, offset=b.offset + K,
                             ap=[b.ap[0], [3 * K, 2], [1, K]])
                a2 = bass.AP(tensor=b.tensor, offset=b.offset + 2 * K,
                             ap=[b.ap[0], [3 * K, 2], [1, K]])
                nc.vector.tensor_add(out=dot2[:], in0=a0, in1=a1)
                nc.vector.tensor_add(out=dot2[:], in0=dot2[:], in1=a2)
                rawuv = dot2[:, 0, :]
                rawvu = dot2[:, 1, :]

                invd = tp.tile([P, K], F16, name="invd")
                nc.vector.reciprocal(out=invd[:], in_=d)
                x = tp.tile([P, K], F16, name="x")
                nc.vector.tensor_scalar(out=x[:], in0=d, scalar1=CUTOFF_SR,
                                        scalar2=1.0 / CUTOFF_SR, op0=A.min,
                                        op1=A.mult)
                maskK = tp.tile([P, K], F16, name="maskK")
                nc.vector.tensor_scalar(out=maskK[:], in0=d, scalar1=CUTOFF,
                                        scalar2=KEHALF, op0=A.is_le,
                                        op1=A.mult)

                # ACT ops depending on DVE
                x2 = tp.tile([P, K], F16, name="x2")
                nc.scalar.activation(out=x2[:], in_=x[:], func=AF.Square)
                u = tp.tile([P, K], F16, name="u")
                nc.scalar.activation(out=u[:], in_=x[:], func=AF.Copy,
                                     bias=-15.0, scale=6.0)
                invd2 = tp.tile([P, K], F16, name="invd2")
                nc.scalar.activation(out=invd2[:], in_=invd[:], func=AF.Square)

                # DVE: reductions from Pool products
                dipdot = tp.tile([P, K], F16, name="dipdot")
                nc.vector.tensor_add(out=dipdot[:], in0=dd3[:, 0, :],
                                     in1=dd3[:, 1, :])
                nc.vector.tensor_add(out=dipdot[:], in0=dipdot[:],
                                     in1=dd3[:, 2, :])
                nc.vector.tensor_add(out=pq6[:, 0:3, :], in0=pq6[:, 0:3, :],
                                     in1=pq6[:, 3:6, :])
                vQv = tp.tile([P, K], F16, name="vQv")
                nc.vector.tensor_add(out=vQv[:], in0=pq6[:, 0, :],
                                     in1=pq6[:, 1, :])
                nc.vector.tensor_add(out=vQv[:], in0=vQv[:], in1=pq6[:, 2, :])

                # DVE: ddinv + switch chain (ACT-coupled)
                ddinv = tp.tile([P, K], F16, name="ddinv")
                nc.vector.reciprocal(out=ddinv[:], in_=ddamp[:])
                nc.vector.tensor_mul(out=u[:], in0=u[:], in1=x[:])      # u2
                u3 = tp.tile([P, K], F16, name="u3")
                nc.scalar.activation(out=u3[:], in_=u[:], func=AF.Copy,
                                     bias=10.0, scale=1.0)
                nc.vector.tensor_mul(out=x2[:], in0=x2[:], in1=x[:])    # x3
                nc.vector.tensor_mul(out=x2[:], in0=x2[:], in1=u3[:])   # p
                Dt = tp.tile([P, K], F16, name="Dt")
                nc.vector.tensor_sub(out=Dt[:], in0=ddinv[:], in1=invd[:])
                nc.vector.tensor_mul(out=x2[:], in0=x2[:], in1=Dt[:])   # p*D
                chi = Dt
                nc.vector.tensor_sub(out=chi[:], in0=ddinv[:], in1=x2[:])

                chi2 = tp.tile([P, K], F16, name="chi2")
                nc.scalar.activation(out=chi2[:], in_=chi[:], func=AF.Square)
                Bd = tp.tile([P, K], F16, name="Bd")
                nc.scalar.activation(out=Bd[:], in_=chi[:], func=AF.Square,
                                     scale=SQRT2)                        # 2chi^2
                nc.vector.tensor_mul(out=chi2[:], in0=chi2[:], in1=chi[:])  # chi3
                Ac = s1
                nc.vector.tensor_add(out=Ac[:], in0=chi[:], in1=s1[:])
                nc.vector.tensor_add(out=Bd[:], in0=Bd[:], in1=s2[:])
                nc.vector.tensor_mul(out=Bd[:], in0=Bd[:], in1=invd[:])
                Cc = s3
                nc.vector.tensor_add(out=Cc[:], in0=chi2[:], in1=s3[:])

                # ---- DVE: assembly (qu factored) ----
                # (vQv - trQ*d2/3)*invd2 == vQv*invd2 - trQ/3  (d2*invd2 = 1)
                tq = trQ
                nc.vector.tensor_scalar(out=tq[:], in0=trQ[:],
                                        scalar1=1.0 / 3.0, scalar2=None,
                                        op0=A.mult)
                vqi = vQv
                nc.vector.tensor_mul(out=vqi[:], in0=vQv[:], in1=invd2[:])
                nc.vector.tensor_sub(out=tq[:], in0=vqi[:], in1=tq[:])
                nc.vector.tensor_mul(out=tq[:], in0=tq[:], in1=Cc[:])   # t4
                t1 = tp.tile([P, K], F16, name="t1")
                nc.vector.tensor_mul(out=t1[:], in0=qv, in1=Ac[:])
                t2 = tp.tile([P, K], F16, name="t2")
                nc.vector.tensor_mul(out=t2[:], in0=rawuv, in1=Bd[:])
                nc.vector.tensor_add(out=t1[:], in0=t1[:], in1=t2[:])
                nc.vector.tensor_add(out=t1[:], in0=t1[:], in1=tq[:])
                nc.vector.tensor_mul(out=t1[:], in0=t1[:], in1=qu)      # qu*(...)
                ttr = t2
                nc.vector.tensor_mul(out=ttr[:], in0=rawuv, in1=rawvu)
                nc.vector.tensor_mul(out=ttr[:], in0=ttr[:], in1=invd2[:])
                tt3 = x2
                nc.vector.tensor_scalar(out=tt3[:], in0=ttr[:], scalar1=3.0,
                                        scalar2=None, op0=A.mult)
                c1s = ttr
                nc.vector.tensor_sub(out=c1s[:], in0=dipdot[:], in1=tt3[:])
                nc.vector.tensor_mul(out=c1s[:], in0=c1s[:], in1=Cc[:])
                nc.vector.tensor_add(out=t1[:], in0=t1[:], in1=c1s[:])  # E
                out_t = io.tile([P, K], F16, name="out_t")
                nc.vector.tensor_mul(out=out_t[:], in0=t1[:], in1=maskK[:])
                nc.sync.dma_start(out=eout[:, it * K:(it + 1) * K],
                                  in_=out_t[:])

            for it in range(NT):
                tile_prog(it)
    nc.compile()
    return nc


def _pack_f16(parts, w, K):
    """parts: list of [E_CORE(,k)] arrays -> [P, NT, w_total, K] f16."""
    NT = COLS // K
    out = np.zeros((P, w, COLS), np.float16)
    pl = 0
    for a in parts:
        if a.ndim == 1:
            out[:, pl, :COLS_REAL] = a.reshape(P, COLS_REAL)
            pl += 1
        else:
            k = a.shape[1]
            out[:, pl:pl + k, :COLS_REAL] = np.moveaxis(
                a.reshape(P, COLS_REAL, k), 2, 1)
            pl += k
    assert pl == w
    # [P, w, COLS] -> [P, NT, w, K]
    return np.ascontiguousarray(np.moveaxis(out.reshape(P, w, NT, K), 2, 1))


def kernel(atomic_charges, atomic_dipoles, atomic_quadrupoles,
           vectors_uv, distances_uv, idx_u, idx_v):
    q = np.ascontiguousarray(np.asarray(atomic_charges, np.float32))
    dip = np.ascontiguousarray(np.asarray(atomic_dipoles, np.float32))
    quad = np.ascontiguousarray(
        np.asarray(atomic_quadrupoles, np.float32)).reshape(-1, 9)
    # columns: diag(0,4,8), upper(1,2,5), lower(3,6,7)
    quad = np.ascontiguousarray(quad[:, [0, 4, 8, 1, 2, 5, 3, 6, 7]])
    vec = np.ascontiguousarray(np.asarray(vectors_uv, np.float32))
    dist = np.ascontiguousarray(np.asarray(distances_uv, np.float32))
    iu = np.asarray(idx_u).astype(np.int64)
    iv = np.asarray(idx_v).astype(np.int64)

    K = _CACHE.get("K", 640)
    if "nc" not in _CACHE:
        _CACHE["nc"] = _build(K=K)
    nc = _CACHE["nc"]

    in_maps = []
    for c in range(N_CORES):
        sl = slice(c * E_CORE, (c + 1) * E_CORE)
        iu_c, iv_c = iu[sl], iv[sl]
        d_c = np.where(dist[sl] == 0, 1.0, dist[sl]).astype(np.float32)
        m = _pack_f16([d_c, vec[sl], q[iu_c], q[iv_c], dip[iu_c], dip[iv_c],
                       quad[iv_c]], NPL, K)
        # pad cols: d -> 5.0 (finite everywhere; qu=0 zeroes the output)
        nt_real, rem = divmod(COLS_REAL, K)
        if rem:
            m[:, nt_real, 0, rem:] = 5.0
        for t in range(nt_real + (1 if rem else 0), COLS // K):
            m[:, t, 0, :] = 5.0
        in_maps.append({"s21_in": m})

    res = run_bass_kernel_spmd(nc, in_maps, core_ids=list(range(N_CORES)))
    _CACHE["last_results"] = res

    out = np.empty(E_TOTAL, np.float32)
    for c in range(N_CORES):
        out[c * E_CORE:(c + 1) * E_CORE] = \
            res.results[c]["eout"][:, :COLS_REAL].astype(np.float32).reshape(-1)
    return out


# revision 14
# speedup vs baseline: 1.5751x; 1.1157x over previous
"""Damped electrostatics (shifted force) TRN2 kernel — v3 (f16, 3-engine).

Strategy:
  - Shard E=3.2M edges across 8 NeuronCores (400K edges each).
  - Host marshals: gathers per-atom records to per-edge streams (np.take,
    pure data movement), casts to f16, packs one planar stream
    s21 = [d, v(3), qu, qv, du(3), dv(3), Qdiag(3), Qup(3), Qlo(3)]
    as [P=128, NT, 21, K] per core (COLS=3200 = 3125 real + pad).
  - Device (per tile): ACT does the unary ops (Square/Sqrt/affine Copy —
    one act-func set); Pool (GPSIMD) does the quadrupole/dipole product
    block via scalar_tensor_tensor (0.6-efficiency path); DVE does the
    rest in f16 fast modes (tensor_tensor 2x, tensor_scalar 4x).
  - Math: chi = ddinv - p*(ddinv - invd), p = x^3(6x^2-15x+10), x=min(d,4)/4;
    E = KEHALF*mask*[qu*(qv*Ac + rawuv*Bd + (vQv - trQ*d2/3)*invd2*Cc)
        + (dipdot - 3*rawuv*rawvu*invd2)*Cc]
    with Ac = chi+0.01d-0.2, Bd = (2chi^2+0.004d-0.06)*invd,
    Cc = chi^3+0.0003d-0.004, rawuv = v.dv, rawvu = v.du.

Self-contained: hardcodes all shapes; no file reads.
"""
import numpy as np

import concourse.bass as bass
import concourse.bacc as bacc
import concourse.tile as tile
from concourse import mybir
from concourse.bass_utils import run_bass_kernel_spmd

F16 = mybir.dt.float16
F32 = mybir.dt.float32

N_CORES = 8
E_TOTAL = 3_200_000
E_CORE = E_TOTAL // N_CORES      # 400_000
P = 128
COLS_REAL = E_CORE // P          # 3125
COLS = 3200                      # padded
NPL = 21                         # input planes per edge

CUTOFF = 10.0
CUTOFF_SR = 4.0
KEHALF = 7.199822675975274
SQRT2 = 1.4142135623730951

_CACHE = {}


def _build(K=640, io_bufs=3, tp_bufs=2):
    NT = COLS // K
    nc = bacc.Bacc("TRN2", target_bir_lowering=False, debug=False,
                   num_devices=N_CORES)
    A = mybir.AluOpType
    AF = mybir.ActivationFunctionType

    s21 = nc.dram_tensor("s21_in", [P, NT, NPL, K], F16, kind="ExternalInput")
    eout = nc.dram_tensor("eout", [P, COLS], F16, kind="ExternalOutput")

    def stt(out, in0, in1, op):
        """Pool binary op (TensorTensor is the only legal Pool ALU opcode)."""
        nc.gpsimd.tensor_tensor(out=out, in0=in0, in1=in1, op=op)

    with tile.TileContext(nc) as tc, \
         nc.allow_low_precision(reason="f16 kernel, rel-err gate 2e-2"):
        with tc.tile_pool(name="io", bufs=io_bufs) as io, \
             tc.tile_pool(name="tp", bufs=tp_bufs) as tp:

            def tile_prog(it):
                st = io.tile([P, NPL, K], F16, name="st")
                nc.sync.dma_start(out=st[:], in_=s21[:, it, :, :])
                d = st[:, 0, :]
                v = st[:, 1:4, :]
                qu = st[:, 4, :]
                qv = st[:, 5, :]
                du = st[:, 6:9, :]
                dv = st[:, 9:12, :]
                Qd = st[:, 12:15, :]
                Qup = st[:, 15:18, :]
                Qlo = st[:, 18:21, :]

                # ---- Pool: product block + trQ (independent of DVE) ----
                dd3 = tp.tile([P, 3, K], F16, name="dd3")
                stt(dd3[:], du, dv, A.mult)
                qoff = tp.tile([P, 3, K], F16, name="qoff")
                stt(qoff[:], Qup, Qlo, A.add)
                vv6 = tp.tile([P, 6, K], F16, name="vv6")
                stt(vv6[:, 0:3, :], v, v, A.mult)
                vx = st[:, 1, :]
                vxb = bass.AP(tensor=vx.tensor, offset=vx.offset,
                              ap=[vx.ap[0], [0, 2], [1, K]])
                nc.vector.tensor_mul(out=vv6[:, 3:5, :], in0=vxb,
                                     in1=st[:, 2:4, :])
                nc.vector.tensor_mul(out=vv6[:, 5, :], in0=st[:, 2, :],
                                     in1=st[:, 3, :])
                pq6 = vv6
                stt(pq6[:, 0:3, :], vv6[:, 0:3, :], Qd, A.mult)
                nc.vector.tensor_mul(out=pq6[:, 3:6, :], in0=vv6[:, 3:6, :],
                                     in1=qoff[:])
                trQ = tp.tile([P, K], F16, name="trQ")
                stt(trQ[:], Qd[:, 0, :], Qd[:, 1, :], A.add)
                stt(trQ[:], trQ[:], Qd[:, 2, :], A.add)

                # ---- ACT: unaries from d ----
                d2 = tp.tile([P, K], F16, name="d2")
                nc.scalar.activation(out=d2[:], in_=d, func=AF.Square)
                ddamp = tp.tile([P, K], F16, name="ddamp")
                nc.scalar.activation(out=ddamp[:], in_=d2[:], func=AF.Sqrt,
                                     bias=1.0, scale=1.0)
                s1 = tp.tile([P, K], F16, name="s1")
                nc.scalar.activation(out=s1[:], in_=d, func=AF.Copy,
                                     bias=-0.2, scale=0.01)
                s2 = tp.tile([P, K], F16, name="s2")
                nc.scalar.activation(out=s2[:], in_=d, func=AF.Copy,
                                     bias=-0.06, scale=0.004)
                s3 = tp.tile([P, K], F16, name="s3")
                nc.scalar.activation(out=s3[:], in_=d, func=AF.Copy,
                                     bias=-0.004, scale=0.0003)

                # ---- DVE: st-dependent work first (overlaps ACT/Pool) ----
                p6 = tp.tile([P, 6, K], F16, name="p6")
                nc.vector.tensor_mul(out=p6[:, 0:3, :], in0=v, in1=dv)
                nc.vector.tensor_mul(out=p6[:, 3:6, :], in0=v, in1=du)
                dot2 = tp.tile([P, 2, K], F16, name="dot2")
                b = p6[:]
                a0 = bass.AP(tensor=b.tensor, offset=b.offset,
                             ap=[b.ap[0], [3 * K, 2], [1, K]])
                a1 = bass.AP(tensor=b.tensor, offset=b.offset + K,
                             ap=[b.ap[0], [3 * K, 2], [1, K]])
                a2 = bass.AP(tensor=b.tensor, offset=b.offset + 2 * K,
                             ap=[b.ap[0], [3 * K, 2], [1, K]])
                nc.vector.tensor_add(out=dot2[:], in0=a0, in1=a1)
                nc.vector.tensor_add(out=dot2[:], in0=dot2[:], in1=a2)
                rawuv = dot2[:, 0, :]
                rawvu = dot2[:, 1, :]

                invd = tp.tile([P, K], F16, name="invd")
                nc.vector.reciprocal(out=invd[:], in_=d)
                x = tp.tile([P, K], F16, name="x")
                nc.vector.tensor_scalar(out=x[:], in0=d, scalar1=CUTOFF_SR,
                                        scalar2=1.0 / CUTOFF_SR, op0=A.min,
                                        op1=A.mult)
                maskK = tp.tile([P, K], F16, name="maskK")
                nc.vector.tensor_scalar(out=maskK[:], in0=d, scalar1=CUTOFF,
                                        scalar2=KEHALF, op0=A.is_le,
                                        op1=A.mult)

                # ACT ops depending on DVE
                x2 = tp.tile([P, K], F16, name="x2")
                nc.scalar.activation(out=x2[:], in_=x[:], func=AF.Square)
                u = tp.tile([P, K], F16, name="u")
                nc.scalar.activation(out=u[:], in_=x[:], func=AF.Copy,
                                     bias=-15.0, scale=6.0)
                invd2 = tp.tile([P, K], F16, name="invd2")
                nc.scalar.activation(out=invd2[:], in_=invd[:], func=AF.Square)

                # DVE: reductions from Pool products
                dipdot = tp.tile([P, K], F16, name="dipdot")
                nc.vector.tensor_add(out=dipdot[:], in0=dd3[:, 0, :],
                                     in1=dd3[:, 1, :])
                nc.vector.tensor_add(out=dipdot[:], in0=dipdot[:],
                                     in1=dd3[:, 2, :])
                nc.vector.tensor_add(out=pq6[:, 0:3, :], in0=pq6[:, 0:3, :],
                                     in1=pq6[:, 3:6, :])
                vQv = tp.tile([P, K], F16, name="vQv")
                nc.vector.tensor_add(out=vQv[:], in0=pq6[:, 0, :],
                                     in1=pq6[:, 1, :])
                nc.vector.tensor_add(out=vQv[:], in0=vQv[:], in1=pq6[:, 2, :])

                # DVE: ddinv + switch chain (ACT-coupled)
                ddinv = tp.tile([P, K], F16, name="ddinv")
                nc.vector.reciprocal(out=ddinv[:], in_=ddamp[:])
                nc.vector.tensor_mul(out=u[:], in0=u[:], in1=x[:])      # u2
                u3 = tp.tile([P, K], F16, name="u3")
                nc.scalar.activation(out=u3[:], in_=u[:], func=AF.Copy,
                                     bias=10.0, scale=1.0)
                nc.vector.tensor_mul(out=x2[:], in0=x2[:], in1=x[:])    # x3
                nc.vector.tensor_mul(out=x2[:], in0=x2[:], in1=u3[:])   # p
                Dt = tp.tile([P, K], F16, name="Dt")
                nc.vector.tensor_sub(out=Dt[:], in0=ddinv[:], in1=invd[:])
                nc.vector.tensor_mul(out=x2[:], in0=x2[:], in1=Dt[:])   # p*D
                chi = Dt
                nc.vector.tensor_sub(out=chi[:], in0=ddinv[:], in1=x2[:])

                chi2 = tp.tile([P, K], F16, name="chi2")
                nc.scalar.activation(out=chi2[:], in_=chi[:], func=AF.Square)
                Bd = tp.tile([P, K], F16, name="Bd")
                nc.scalar.activation(out=Bd[:], in_=chi[:], func=AF.Square,
                                     scale=SQRT2)                        # 2chi^2
                nc.vector.tensor_mul(out=chi2[:], in0=chi2[:], in1=chi[:])  # chi3
                Ac = s1
                nc.vector.tensor_add(out=Ac[:], in0=chi[:], in1=s1[:])
                nc.vector.tensor_add(out=Bd[:], in0=Bd[:], in1=s2[:])
                nc.vector.tensor_mul(out=Bd[:], in0=Bd[:], in1=invd[:])
                Cc = s3
                nc.vector.tensor_add(out=Cc[:], in0=chi2[:], in1=s3[:])

                # ---- DVE: assembly (qu factored) ----
                # (vQv - trQ*d2/3)*invd2 == vQv*invd2 - trQ/3  (d2*invd2 = 1)
                tq = trQ
                nc.vector.tensor_scalar(out=tq[:], in0=trQ[:],
                                        scalar1=1.0 / 3.0, scalar2=None,
                                        op0=A.mult)
                vqi = vQv
                nc.vector.tensor_mul(out=vqi[:], in0=vQv[:], in1=invd2[:])
                nc.vector.tensor_sub(out=tq[:], in0=vqi[:], in1=tq[:])
                nc.vector.tensor_mul(out=tq[:], in0=tq[:], in1=Cc[:])   # t4
                t1 = tp.tile([P, K], F16, name="t1")
                nc.vector.tensor_mul(out=t1[:], in0=qv, in1=Ac[:])
                t2 = tp.tile([P, K], F16, name="t2")
                nc.vector.tensor_mul(out=t2[:], in0=rawuv, in1=Bd[:])
                nc.vector.tensor_add(out=t1[:], in0=t1[:], in1=t2[:])
                nc.vector.tensor_add(out=t1[:], in0=t1[:], in1=tq[:])
                nc.vector.tensor_mul(out=t1[:], in0=t1[:], in1=qu)      # qu*(...)
                ttr = t2
                nc.vector.tensor_mul(out=ttr[:], in0=rawuv, in1=rawvu)
                nc.vector.tensor_mul(out=ttr[:], in0=ttr[:], in1=invd2[:])
                tt3 = x2
                nc.vector.tensor_scalar(out=tt3[:], in0=ttr[:], scalar1=3.0,
                                        scalar2=None, op0=A.mult)
                c1s = ttr
                nc.vector.tensor_sub(out=c1s[:], in0=dipdot[:], in1=tt3[:])
                nc.vector.tensor_mul(out=c1s[:], in0=c1s[:], in1=Cc[:])
                nc.vector.tensor_add(out=t1[:], in0=t1[:], in1=c1s[:])  # E
                out_t = io.tile([P, K], F16, name="out_t")
                nc.vector.tensor_mul(out=out_t[:], in0=t1[:], in1=maskK[:])
                nc.sync.dma_start(out=eout[:, it * K:(it + 1) * K],
                                  in_=out_t[:])

            for it in range(NT):
                tile_prog(it)
    nc.compile()
    return nc


def _pack_f16(parts, w, K):
    """parts: list of [E_CORE(,k)] arrays -> [P, NT, w_total, K] f16."""
    NT = COLS // K
    out = np.zeros((P, w, COLS), np.float16)
    pl = 0
    for a in parts:
        if a.ndim == 1:
            out[:, pl, :COLS_REAL] = a.reshape(P, COLS_REAL)
            pl += 1
        else:
            k = a.shape[1]
            out[:, pl:pl + k, :COLS_REAL] = np.moveaxis(
                a.reshape(P, COLS_REAL, k), 2, 1)
            pl += k
    assert pl == w
    # [P, w, COLS] -> [P, NT, w, K]
    return np.ascontiguousarray(np.moveaxis(out.reshape(P, w, NT, K), 2, 1))


def kernel(atomic_charges, atomic_dipoles, atomic_quadrupoles,
           vectors_uv, distances_uv, idx_u, idx_v):
    q = np.ascontiguousarray(np.asarray(atomic_charges, np.float32))
    dip = np.ascontiguousarray(np.asarray(atomic_dipoles, np.float32))
    quad = np.ascontiguousarray(
        np.asarray(atomic_quadrupoles, np.float32)).reshape(-1, 9)
    # columns: diag(0,4,8), upper(1,2,5), lower(3,6,7)
    quad = np.ascontiguousarray(quad[:, [0, 4, 8, 1, 2, 5, 3, 6, 7]])
    vec = np.ascontiguousarray(np.asarray(vectors_uv, np.float32))
    dist = np.ascontiguousarray(np.asarray(distances_uv, np.float32))
    iu = np.asarray(idx_u).astype(np.int64)
    iv = np.asarray(idx_v).astype(np.int64)

    K = _CACHE.get("K", 640)
    if "nc" not in _CACHE:
        _CACHE["nc"] = _build(K=K)
    nc = _CACHE["nc"]

    in_maps = []
    for c in range(N_CORES):
        sl = slice(c * E_CORE, (c + 1) * E_CORE)
        iu_c, iv_c = iu[sl], iv[sl]
        d_c = np.where(dist[sl] == 0, 1.0, dist[sl]).astype(np.float32)
        m = _pack_f16([d_c, vec[sl], q[iu_c], q[iv_c], dip[iu_c], dip[iv_c],
                       quad[iv_c]], NPL, K)
        # pad cols: d -> 5.0 (finite everywhere; qu=0 zeroes the output)
        nt_real, rem = divmod(COLS_REAL, K)
        if rem:
            m[:, nt_real, 0, rem:] = 5.0
        for t in range(nt_real + (1 if rem else 0), COLS // K):
            m[:, t, 0, :] = 5.0
        in_maps.append({"s21_in": m})

    res = run_bass_kernel_spmd(nc, in_maps, core_ids=list(range(N_CORES)))
    _CACHE["last_results"] = res

    out = np.empty(E_TOTAL, np.float32)
    for c in range(N_CORES):
        out[c * E_CORE:(c + 1) * E_CORE] = \
            res.results[c]["eout"][:, :COLS_REAL].astype(np.float32).reshape(-1)
    return out


# revision 15
# speedup vs baseline: 1.6342x; 1.0375x over previous
"""Damped electrostatics (shifted force) TRN2 kernel — v3 (f16, 3-engine).

Strategy:
  - Shard E=3.2M edges across 8 NeuronCores (400K edges each).
  - Host marshals: gathers per-atom records to per-edge streams (np.take,
    pure data movement), casts to f16, packs one planar stream
    s21 = [d, v(3), qu, qv, du(3), dv(3), Qdiag(3), Qup(3), Qlo(3)]
    as [P=128, NT, 21, K] per core (COLS=3200 = 3125 real + pad).
  - Device (per tile): ACT does the unary ops (Square/Sqrt/affine Copy —
    one act-func set); Pool (GPSIMD) does the quadrupole/dipole product
    block via scalar_tensor_tensor (0.6-efficiency path); DVE does the
    rest in f16 fast modes (tensor_tensor 2x, tensor_scalar 4x).
  - Math: chi = ddinv - p*(ddinv - invd), p = x^3(6x^2-15x+10), x=min(d,4)/4;
    E = KEHALF*mask*[qu*(qv*Ac + rawuv*Bd + (vQv - trQ*d2/3)*invd2*Cc)
        + (dipdot - 3*rawuv*rawvu*invd2)*Cc]
    with Ac = chi+0.01d-0.2, Bd = (2chi^2+0.004d-0.06)*invd,
    Cc = chi^3+0.0003d-0.004, rawuv = v.dv, rawvu = v.du.

Self-contained: hardcodes all shapes; no file reads.
"""
import numpy as np

import concourse.bass as bass
import concourse.bacc as bacc
import concourse.tile as tile
from concourse import mybir
from concourse.bass_utils import run_bass_kernel_spmd

F16 = mybir.dt.float16
F32 = mybir.dt.float32

N_CORES = 8
E_TOTAL = 3_200_000
E_CORE = E_TOTAL // N_CORES      # 400_000
P = 128
COLS_REAL = E_CORE // P          # 3125
COLS = 3200                      # padded
NPL = 21                         # input planes per edge

CUTOFF = 10.0
CUTOFF_SR = 4.0
KEHALF = 7.199822675975274
SQRT2 = 1.4142135623730951

_CACHE = {}


def _build(K=640, io_bufs=3, tp_bufs=2):
    NT = COLS // K
    nc = bacc.Bacc("TRN2", target_bir_lowering=False, debug=False,
                   num_devices=N_CORES)
    A = mybir.AluOpType
    AF = mybir.ActivationFunctionType

    s21 = nc.dram_tensor("s21_in", [P, NT, NPL, K], F16, kind="ExternalInput")
    eout = nc.dram_tensor("eout", [P, COLS], F16, kind="ExternalOutput")

    def stt(out, in0, in1, op):
        """Pool binary op (TensorTensor is the only legal Pool ALU opcode)."""
        nc.gpsimd.tensor_tensor(out=out, in0=in0, in1=in1, op=op)

    with tile.TileContext(nc) as tc, \
         nc.allow_low_precision(reason="f16 kernel, rel-err gate 2e-2"):
        with tc.tile_pool(name="io", bufs=io_bufs) as io, \
             tc.tile_pool(name="tp", bufs=tp_bufs) as tp:

            def tile_prog(it):
                st = io.tile([P, NPL, K], F16, name="st")
                # hot planes (d,v,qu,qv) first so compute starts early
                nc.sync.dma_start(out=st[:, 0:6, :], in_=s21[:, it, 0:6, :])
                nc.sync.dma_start(out=st[:, 6:12, :], in_=s21[:, it, 6:12, :])
                nc.sync.dma_start(out=st[:, 12:21, :], in_=s21[:, it, 12:21, :])
                d = st[:, 0, :]
                v = st[:, 1:4, :]
                qu = st[:, 4, :]
                qv = st[:, 5, :]
                du = st[:, 6:9, :]
                dv = st[:, 9:12, :]
                Qd = st[:, 12:15, :]
                Qup = st[:, 15:18, :]
                Qlo = st[:, 18:21, :]

                # ---- Pool: product block + trQ (independent of DVE) ----
                dd3 = tp.tile([P, 3, K], F16, name="dd3")
                stt(dd3[:], du, dv, A.mult)
                qoff = tp.tile([P, 3, K], F16, name="qoff")
                stt(qoff[:], Qup, Qlo, A.add)
                vv6 = tp.tile([P, 6, K], F16, name="vv6")
                stt(vv6[:, 0:3, :], v, v, A.mult)
                vx = st[:, 1, :]
                vxb = bass.AP(tensor=vx.tensor, offset=vx.offset,
                              ap=[vx.ap[0], [0, 2], [1, K]])
                nc.vector.tensor_mul(out=vv6[:, 3:5, :], in0=vxb,
                                     in1=st[:, 2:4, :])
                nc.vector.tensor_mul(out=vv6[:, 5, :], in0=st[:, 2, :],
                                     in1=st[:, 3, :])
                pq6 = vv6
                stt(pq6[:, 0:3, :], vv6[:, 0:3, :], Qd, A.mult)
                nc.vector.tensor_mul(out=pq6[:, 3:6, :], in0=vv6[:, 3:6, :],
                                     in1=qoff[:])
                trQ = tp.tile([P, K], F16, name="trQ")
                stt(trQ[:], Qd[:, 0, :], Qd[:, 1, :], A.add)
                stt(trQ[:], trQ[:], Qd[:, 2, :], A.add)

                # ---- ACT: unaries from d ----
                d2 = tp.tile([P, K], F16, name="d2")
                nc.scalar.activation(out=d2[:], in_=d, func=AF.Square)
                ddamp = tp.tile([P, K], F16, name="ddamp")
                nc.scalar.activation(out=ddamp[:], in_=d2[:], func=AF.Sqrt,
                                     bias=1.0, scale=1.0)
                s1 = tp.tile([P, K], F16, name="s1")
                nc.scalar.activation(out=s1[:], in_=d, func=AF.Copy,
                                     bias=-0.2, scale=0.01)
                s2 = tp.tile([P, K], F16, name="s2")
                nc.scalar.activation(out=s2[:], in_=d, func=AF.Copy,
                                     bias=-0.06, scale=0.004)
                s3 = tp.tile([P, K], F16, name="s3")
                nc.scalar.activation(out=s3[:], in_=d, func=AF.Copy,
                                     bias=-0.004, scale=0.0003)

                # ---- DVE: st-dependent work first (overlaps ACT/Pool) ----
                p6 = tp.tile([P, 6, K], F16, name="p6")
                nc.vector.tensor_mul(out=p6[:, 0:3, :], in0=v, in1=dv)
                nc.vector.tensor_mul(out=p6[:, 3:6, :], in0=v, in1=du)
                dot2 = tp.tile([P, 2, K], F16, name="dot2")
                b = p6[:]
                a0 = bass.AP(tensor=b.tensor, offset=b.offset,
                             ap=[b.ap[0], [3 * K, 2], [1, K]])
                a1 = bass.AP(tensor=b.tensor, offset=b.offset + K,
                             ap=[b.ap[0], [3 * K, 2], [1, K]])
                a2 = bass.AP(tensor=b.tensor, offset=b.offset + 2 * K,
                             ap=[b.ap[0], [3 * K, 2], [1, K]])
                nc.vector.tensor_add(out=dot2[:], in0=a0, in1=a1)
                nc.vector.tensor_add(out=dot2[:], in0=dot2[:], in1=a2)
                rawuv = dot2[:, 0, :]
                rawvu = dot2[:, 1, :]

                invd = tp.tile([P, K], F16, name="invd")
                nc.vector.reciprocal(out=invd[:], in_=d)
                x = tp.tile([P, K], F16, name="x")
                nc.vector.tensor_scalar(out=x[:], in0=d, scalar1=CUTOFF_SR,
                                        scalar2=1.0 / CUTOFF_SR, op0=A.min,
                                        op1=A.mult)
                maskK = tp.tile([P, K], F16, name="maskK")
                nc.vector.tensor_scalar(out=maskK[:], in0=d, scalar1=CUTOFF,
                                        scalar2=KEHALF, op0=A.is_le,
                                        op1=A.mult)

                # ACT ops depending on DVE
                x2 = tp.tile([P, K], F16, name="x2")
                nc.scalar.activation(out=x2[:], in_=x[:], func=AF.Square)
                u = tp.tile([P, K], F16, name="u")
                nc.scalar.activation(out=u[:], in_=x[:], func=AF.Copy,
                                     bias=-15.0, scale=6.0)
                invd2 = tp.tile([P, K], F16, name="invd2")
                nc.scalar.activation(out=invd2[:], in_=invd[:], func=AF.Square)

                # DVE: reductions from Pool products
                dipdot = tp.tile([P, K], F16, name="dipdot")
                nc.vector.tensor_add(out=dipdot[:], in0=dd3[:, 0, :],
                                     in1=dd3[:, 1, :])
                nc.vector.tensor_add(out=dipdot[:], in0=dipdot[:],
                                     in1=dd3[:, 2, :])
                nc.vector.tensor_add(out=pq6[:, 0:3, :], in0=pq6[:, 0:3, :],
                                     in1=pq6[:, 3:6, :])
                vQv = tp.tile([P, K], F16, name="vQv")
                nc.vector.tensor_add(out=vQv[:], in0=pq6[:, 0, :],
                                     in1=pq6[:, 1, :])
                nc.vector.tensor_add(out=vQv[:], in0=vQv[:], in1=pq6[:, 2, :])

                # DVE: ddinv + switch chain (ACT-coupled)
                ddinv = tp.tile([P, K], F16, name="ddinv")
                nc.vector.reciprocal(out=ddinv[:], in_=ddamp[:])
                nc.vector.tensor_mul(out=u[:], in0=u[:], in1=x[:])      # u2
                u3 = tp.tile([P, K], F16, name="u3")
                nc.scalar.activation(out=u3[:], in_=u[:], func=AF.Copy,
                                     bias=10.0, scale=1.0)
                nc.vector.tensor_mul(out=x2[:], in0=x2[:], in1=x[:])    # x3
                nc.vector.tensor_mul(out=x2[:], in0=x2[:], in1=u3[:])   # p
                Dt = tp.tile([P, K], F16, name="Dt")
                nc.vector.tensor_sub(out=Dt[:], in0=ddinv[:], in1=invd[:])
                nc.vector.tensor_mul(out=x2[:], in0=x2[:], in1=Dt[:])   # p*D
                chi = Dt
                nc.vector.tensor_sub(out=chi[:], in0=ddinv[:], in1=x2[:])

                chi2 = tp.tile([P, K], F16, name="chi2")
                nc.scalar.activation(out=chi2[:], in_=chi[:], func=AF.Square)
                Bd = tp.tile([P, K], F16, name="Bd")
                nc.scalar.activation(out=Bd[:], in_=chi[:], func=AF.Square,
                                     scale=SQRT2)                        # 2chi^2
                nc.vector.tensor_mul(out=chi2[:], in0=chi2[:], in1=chi[:])  # chi3
                Ac = s1
                nc.vector.tensor_add(out=Ac[:], in0=chi[:], in1=s1[:])
                nc.vector.tensor_add(out=Bd[:], in0=Bd[:], in1=s2[:])
                nc.vector.tensor_mul(out=Bd[:], in0=Bd[:], in1=invd[:])
                Cc = s3
                nc.vector.tensor_add(out=Cc[:], in0=chi2[:], in1=s3[:])

                # ---- DVE: assembly (qu factored) ----
                # (vQv - trQ*d2/3)*invd2 == vQv*invd2 - trQ/3  (d2*invd2 = 1)
                tq = trQ
                nc.vector.tensor_scalar(out=tq[:], in0=trQ[:],
                                        scalar1=1.0 / 3.0, scalar2=None,
                                        op0=A.mult)
                vqi = vQv
                nc.vector.tensor_mul(out=vqi[:], in0=vQv[:], in1=invd2[:])
                nc.vector.tensor_sub(out=tq[:], in0=vqi[:], in1=tq[:])
                nc.vector.tensor_mul(out=tq[:], in0=tq[:], in1=Cc[:])   # t4
                t1 = tp.tile([P, K], F16, name="t1")
                nc.vector.tensor_mul(out=t1[:], in0=qv, in1=Ac[:])
                t2 = tp.tile([P, K], F16, name="t2")
                nc.vector.tensor_mul(out=t2[:], in0=rawuv, in1=Bd[:])
                nc.vector.tensor_add(out=t1[:], in0=t1[:], in1=t2[:])
                nc.vector.tensor_add(out=t1[:], in0=t1[:], in1=tq[:])
                nc.vector.tensor_mul(out=t1[:], in0=t1[:], in1=qu)      # qu*(...)
                ttr = t2
                nc.vector.tensor_mul(out=ttr[:], in0=rawuv, in1=rawvu)
                nc.vector.tensor_mul(out=ttr[:], in0=ttr[:], in1=invd2[:])
                tt3 = x2
                nc.vector.tensor_scalar(out=tt3[:], in0=ttr[:], scalar1=3.0,
                                        scalar2=None, op0=A.mult)
                c1s = ttr
                nc.vector.tensor_sub(out=c1s[:], in0=dipdot[:], in1=tt3[:])
                nc.vector.tensor_mul(out=c1s[:], in0=c1s[:], in1=Cc[:])
                nc.vector.tensor_add(out=t1[:], in0=t1[:], in1=c1s[:])  # E
                out_t = io.tile([P, K], F16, name="out_t")
                nc.vector.tensor_mul(out=out_t[:], in0=t1[:], in1=maskK[:])
                nc.sync.dma_start(out=eout[:, it * K:(it + 1) * K],
                                  in_=out_t[:])

            for it in range(NT):
                tile_prog(it)
    nc.compile()
    return nc


def _pack_f16(parts, w, K):
    """parts: list of [E_CORE(,k)] arrays -> [P, NT, w_total, K] f16."""
    NT = COLS // K
    out = np.zeros((P, w, COLS), np.float16)
    pl = 0
    for a in parts:
        if a.ndim == 1:
            out[:, pl, :COLS_REAL] = a.reshape(P, COLS_REAL)
            pl += 1
        else:
            k = a.shape[1]
            out[:, pl:pl + k, :COLS_REAL] = np.moveaxis(
                a.reshape(P, COLS_REAL, k), 2, 1)
            pl += k
    assert pl == w
    # [P, w, COLS] -> [P, NT, w, K]
    return np.ascontiguousarray(np.moveaxis(out.reshape(P, w, NT, K), 2, 1))


def kernel(atomic_charges, atomic_dipoles, atomic_quadrupoles,
           vectors_uv, distances_uv, idx_u, idx_v):
    q = np.ascontiguousarray(np.asarray(atomic_charges, np.float32))
    dip = np.ascontiguousarray(np.asarray(atomic_dipoles, np.float32))
    quad = np.ascontiguousarray(
        np.asarray(atomic_quadrupoles, np.float32)).reshape(-1, 9)
    # columns: diag(0,4,8), upper(1,2,5), lower(3,6,7)
    quad = np.ascontiguousarray(quad[:, [0, 4, 8, 1, 2, 5, 3, 6, 7]])
    vec = np.ascontiguousarray(np.asarray(vectors_uv, np.float32))
    dist = np.ascontiguousarray(np.asarray(distances_uv, np.float32))
    iu = np.asarray(idx_u).astype(np.int64)
    iv = np.asarray(idx_v).astype(np.int64)

    K = _CACHE.get("K", 640)
    if "nc" not in _CACHE:
        _CACHE["nc"] = _build(K=K)
    nc = _CACHE["nc"]

    in_maps = []
    for c in range(N_CORES):
        sl = slice(c * E_CORE, (c + 1) * E_CORE)
        iu_c, iv_c = iu[sl], iv[sl]
        d_c = np.where(dist[sl] == 0, 1.0, dist[sl]).astype(np.float32)
        m = _pack_f16([d_c, vec[sl], q[iu_c], q[iv_c], dip[iu_c], dip[iv_c],
                       quad[iv_c]], NPL, K)
        # pad cols: d -> 5.0 (finite everywhere; qu=0 zeroes the output)
        nt_real, rem = divmod(COLS_REAL, K)
        if rem:
            m[:, nt_real, 0, rem:] = 5.0
        for t in range(nt_real + (1 if rem else 0), COLS // K):
            m[:, t, 0, :] = 5.0
        in_maps.append({"s21_in": m})

    res = run_bass_kernel_spmd(nc, in_maps, core_ids=list(range(N_CORES)))
    _CACHE["last_results"] = res

    out = np.empty(E_TOTAL, np.float32)
    for c in range(N_CORES):
        out[c * E_CORE:(c + 1) * E_CORE] = \
            res.results[c]["eout"][:, :COLS_REAL].astype(np.float32).reshape(-1)
    return out
